# revision 23
# baseline (speedup 1.0000x reference)
"""AverageSpanExtractor Trainium2 kernel (SWDGE gather + fp16 local table).

Math: out[b, n, :] = mean(seq[b, s_n:e_n, :]) * mask[b, n]

Per core (data-parallel over batch across 8 cores):
  1. Load seq [S=2048, D=512] f32, cast fp16.
  2. Per 128-token block: block-diag strict-upper matmul gives the 32-row
     SUB-BLOCK-local exclusive prefix L[r] = sum(seq[32*(r>>5) .. r)); cast
     fp16 and store to a DRAM table [2048, 512] fp16 (|L| < ~45 keeps fp16
     rounding well inside the 2e-2 gate).
  3. Gather tbl[e_n] and tbl[s_n] (raw indices -- exclusive table) with four
     512-idx SWDGE dma_gathers.  The mlp ucode library load (~28us) dominates
     the front; preps (~5us each) interleave with per-gather triggers so the
     transfers and combine overlap descriptor generation.
  4. Missing inter-sub-block offsets C[b] = sum of sub-block totals [0, b)
     are added exactly via a +-onehot fp16 matmul (onehot over e>>5 / s>>5
     from threshold compares on a rank-1 PSUM broadcast of the span indices)
     against hi/lo-split C vectors.
  5. Per span tile: PSUM = onehot@C;  DVE adds tbl[e]-tbl[s];  one ACT pass
     scales by mask/width; store.
"""

import numpy as np

import concourse.bacc as bacc
import concourse.bass as bass
import concourse.tile as tile
from concourse import mybir
from concourse.bass import AP
from concourse.library_config import mlp
from concourse.tile_rust import add_dep_helper

# Problem shape (hardcoded per contract).
B, S, D, N = 8, 2048, 512, 1024
NBLK = S // 128          # 16 token blocks (matmul granularity)
SUB = 32                 # sub-block rows (fp16 table locality)
NSUB = S // SUB          # 64 sub-blocks (offset-table entries)
SPB = 128 // SUB         # 4 sub-blocks per 128-block
NTILE = N // 128         # 8 span tiles
NGATHER = 2              # gather instructions (4 span tiles each)

F32 = mybir.dt.float32
F16 = mybir.dt.float16
I32 = mybir.dt.int32
I16 = mybir.dt.int16


def build_kernel_body(tc: tile.TileContext, seq: AP, spans: AP, maskw: AP,
                      out: AP, tbl, consts, ctx, dbg=None):
    nc = tc.nc
    const = ctx.enter_context(tc.tile_pool(name="const", bufs=1))
    sbuf = ctx.enter_context(tc.tile_pool(name="sbuf", bufs=1))
    opool = ctx.enter_context(tc.tile_pool(name="opool", bufs=3))
    gpool = ctx.enter_context(tc.tile_pool(name="gpool", bufs=1))
    psum_b = ctx.enter_context(tc.tile_pool(name="pb", bufs=2, space="PSUM"))
    psum_c = ctx.enter_context(tc.tile_pool(name="pc", bufs=2, space="PSUM"))
    psum_bc = ctx.enter_context(tc.tile_pool(name="pbc", bufs=1, space="PSUM"))

    # library load first: ~28us on the Pool engine, overlaps everything
    nc.gpsimd.load_library(mlp)

    # constants blob first (needed by the first matmul), then the seq halves
    # go straight onto the sync queue; all other tiny loads queue after them.
    cb = const.tile([128, 576], F16, tag="cb")
    nc.sync.dma_start(cb[:], consts["blob"].ap())
    u_strict = cb[:, 0:128]
    idp = cb[:, 128:256]
    u64s = cb[0:64, 384:448]
    ones1 = cb[0:1, 448:576]

    # ---------------- seq load + fp16 cast + local cumsum + table store ----
    # (seq chunk DMAs queue behind the tiny loads above: first chunk lands
    # ~4.5us, last ~13.5us; stores complete ~17us -- all before the library)
    store_insts = []
    xbig = sbuf.tile([128, NBLK, D], F32, tag="xbig")
    xf = sbuf.tile([128, NBLK, D], F16, tag="xf")
    tbl_sb = sbuf.tile([128, NBLK, D], F16, tag="tbl_sb")
    for h in range(2):
        sl = (slice(None), slice(8 * h, 8 * h + 8), slice(None))
        nc.sync.dma_start(
            xbig[sl],
            seq[1024 * h:1024 * (h + 1), :].rearrange("(j p) d -> p j d", p=128))
        for q in range(2 * h, 2 * h + 2):
            slq = (slice(None), slice(4 * q, 4 * q + 4), slice(None))
            nc.vector.tensor_copy(xf[slq], xbig[slq])
            for b in range(4 * q, 4 * q + 4):
                pl = psum_b.tile([128, D], F32, tag="pb")
                nc.tensor.matmul(out=pl[:], lhsT=u_strict, rhs=xf[:, b, :],
                                 start=True, stop=True)
                if b % 2 == 0:
                    nc.scalar.copy(tbl_sb[:, b, :], pl[:])
                else:
                    nc.vector.tensor_copy(tbl_sb[:, b, :], pl[:])
            # store 4 blocks (512 rows) to the DRAM table, scalar queue
            store_insts.append(nc.scalar.dma_start(
                AP(tbl, 512 * D * q, [[D, 128], [128 * D, 4], [1, D]]),
                tbl_sb[:, 4 * q:4 * q + 4, :]))

    # ---------------- gather idx list + onehot inputs (post-build trace) ---
    # a32[p, 32t + u (+16)] = (e|s) of span 256t + 16u + p; these tiny sync
    # DMAs queue behind the seq chunks and land ~14us -- needed only at the
    # ~35us library-ready point.
    st_pj = sbuf.tile([128, NTILE], I32, tag="st_pj")
    en_pj = sbuf.tile([128, NTILE], I32, tag="en_pj")
    mk_pj = sbuf.tile([128, NTILE], I32, tag="mk_pj")
    nc.sync.dma_start(st_pj[:], AP(spans.tensor, 0, [[2, 128], [256, NTILE]]))
    nc.sync.dma_start(en_pj[:], AP(spans.tensor, 1, [[2, 128], [256, NTILE]]))
    nc.sync.dma_start(mk_pj[:], AP(maskw.tensor, 0, [[1, 128], [128, NTILE]]))
    thr2 = const.tile([NSUB, 2], F32, tag="thr2")
    nc.sync.dma_start(thr2[:], consts["thr2"].ap())
    thr_lo = thr2[:, 0:1]
    thr_hi = thr2[:, 1:2]
    aE = sbuf.tile([16, 64], I32, tag="aE")
    aS = sbuf.tile([16, 64], I32, tag="aS")
    nc.sync.dma_start(aE[:], AP(spans.tensor, 1, [[2, 16], [32, 64]]))
    nc.sync.dma_start(aS[:], AP(spans.tensor, 0, [[2, 16], [32, 64]]))
    idx16 = sbuf.tile([128, 128], I16, tag="idx16")
    i16v = idx16[0:16, :].rearrange("p (t f u) -> p t f u", t=NGATHER, f=2)
    nc.vector.tensor_copy(i16v[:, :, 0, :],
                          aE[:].rearrange("p (t u) -> p t u", t=NGATHER))
    nc.vector.tensor_copy(i16v[:, :, 1, :],
                          aS[:].rearrange("p (t u) -> p t u", t=NGATHER))
    nc.sync.dma_start(idx16[16:32, :], idx16[0:16, :])
    nc.sync.dma_start(idx16[32:64, :], idx16[0:32, :])
    nc.sync.dma_start(idx16[64:128, :], idx16[0:64, :])

    # span indices to fp16, PE transpose, fold to a [1, 2048] row, rank-1
    # broadcast into PSUM (compare ops read it later, after the build)
    es16 = sbuf.tile([128, 2 * NTILE], F16, tag="es16")
    nc.vector.tensor_copy(es16[:, 0:NTILE], en_pj[:])
    nc.vector.tensor_copy(es16[:, NTILE:2 * NTILE], st_pj[:])
    psT = psum_c.tile([2 * NTILE, 128], F16, tag="pc")
    nc.tensor.transpose(out=psT[:], in_=es16[:], identity=idp)
    esT = sbuf.tile([2 * NTILE, 128], F16, tag="esT")
    nc.vector.tensor_copy(esT[:], psT[:])
    esrow = sbuf.tile([1, 2 * N], F16, tag="esrow")
    nc.sync.dma_start(esrow[0:1, 0:N].rearrange("p (j c) -> p j c", j=NTILE),
                      esT[0:NTILE, :])
    nc.sync.dma_start(esrow[0:1, N:2 * N].rearrange("p (j c) -> p j c", j=NTILE),
                      esT[NTILE:2 * NTILE, :])
    bce = psum_bc.tile([128, N], F32, tag="bc")
    nc.tensor.matmul(out=bce[:, 0:D], lhsT=ones1, rhs=esrow[0:1, 0:D],
                     start=True, stop=True)
    nc.tensor.matmul(out=bce[:, D:N], lhsT=ones1, rhs=esrow[0:1, D:N],
                     start=True, stop=True)
    bcs = psum_bc.tile([128, N], F32, tag="bc2")
    nc.tensor.matmul(out=bcs[:, 0:D], lhsT=ones1, rhs=esrow[0:1, N:N + D],
                     start=True, stop=True)
    nc.tensor.matmul(out=bcs[:, D:N], lhsT=ones1, rhs=esrow[0:1, N + D:2 * N],
                     start=True, stop=True)

    # ---------------- onehot rows (DVE compares, after the build casts) ----
    cme = sbuf.tile([NSUB, N], F32, tag="cme")
    nc.vector.tensor_scalar(out=cme[:], in0=bce[0:NSUB, :], scalar1=thr_lo,
                            scalar2=None, op0=mybir.AluOpType.is_ge)
    tmp = sbuf.tile([NSUB, N], F32, tag="tmp")
    nc.vector.tensor_scalar(out=tmp[:], in0=bce[0:NSUB, :], scalar1=thr_hi,
                            scalar2=None, op0=mybir.AluOpType.is_ge)
    nc.vector.tensor_tensor(out=cme[:], in0=cme[:], in1=tmp[:],
                            op=mybir.AluOpType.subtract)
    cms = sbuf.tile([NSUB, N], F32, tag="cms")
    nc.vector.tensor_scalar(out=cms[:], in0=bcs[0:NSUB, :], scalar1=thr_lo,
                            scalar2=None, op0=mybir.AluOpType.is_ge)
    nc.vector.tensor_scalar(out=tmp[:], in0=bcs[0:NSUB, :], scalar1=thr_hi,
                            scalar2=None, op0=mybir.AluOpType.is_ge)
    nc.vector.tensor_tensor(out=cms[:], in0=cms[:], in1=tmp[:],
                            op=mybir.AluOpType.subtract)
    oh = sbuf.tile([128, N], F16, tag="oh")
    nc.vector.tensor_tensor(out=oh[0:NSUB, :], in0=cme[:], in1=cms[:],
                            op=mybir.AluOpType.subtract)
    nc.scalar.dma_start(oh[NSUB:128, :], oh[0:NSUB, :])

    # per-span scale = mask / width
    w_i = sbuf.tile([128, NTILE], I32, tag="w_i")
    nc.vector.tensor_tensor(out=w_i[:], in0=en_pj[:], in1=st_pj[:],
                            op=mybir.AluOpType.subtract)
    w_f = sbuf.tile([128, NTILE], F32, tag="w_f")
    nc.vector.tensor_copy(w_f[:], w_i[:])
    r_f = sbuf.tile([128, NTILE], F32, tag="r_f")
    nc.vector.reciprocal(r_f[:], w_f[:])
    m_f = sbuf.tile([128, NTILE], F32, tag="m_f")
    nc.vector.tensor_copy(m_f[:], mk_pj[:])
    scale = sbuf.tile([128, NTILE], F32, tag="scale")
    nc.vector.tensor_tensor(out=scale[:], in0=r_f[:], in1=m_f[:],
                            op=mybir.AluOpType.mult)

    # ---------------- sub-block totals -> offsets C ------------------------
    # T_b = L[32b + 31] + x[32b + 31]; PERMUTED row order k' = 16i + blk,
    # matching u64s whose rows are permuted (columns in true order b).
    t16f = sbuf.tile([NSUB, D], F16, tag="t16f")
    xrow = sbuf.tile([NSUB, D], F16, tag="xrow")
    for i in range(SPB):
        p = SUB * i + SUB - 1
        nc.scalar.dma_start(t16f[16 * i:16 * (i + 1), :], tbl_sb[p:p + 1, :, :])
        nc.scalar.dma_start(xrow[16 * i:16 * (i + 1), :], xf[p:p + 1, :, :])
    t16 = sbuf.tile([NSUB, D], F32, tag="t16")
    nc.vector.tensor_tensor(out=t16[:], in0=t16f[:], in1=xrow[:],
                            op=mybir.AluOpType.add)
    th = sbuf.tile([NSUB, D], F16, tag="th")
    nc.vector.tensor_copy(th[:], t16[:])
    tl = sbuf.tile([NSUB, D], F16, tag="tl")
    nc.vector.tensor_tensor(out=tl[:], in0=t16[:], in1=th[:],
                            op=mybir.AluOpType.subtract)
    poff = psum_c.tile([NSUB, D], F32, tag="pc")
    nc.tensor.matmul(out=poff[:], lhsT=u64s, rhs=th[:], start=True, stop=False)
    nc.tensor.matmul(out=poff[:], lhsT=u64s, rhs=tl[:], start=False, stop=True)
    cf = sbuf.tile([NSUB, D], F32, tag="cf")
    nc.vector.tensor_copy(cf[:], poff[:])
    chi = sbuf.tile([NSUB, D], F16, tag="chi")
    nc.vector.tensor_copy(chi[:], cf[:])
    clo = sbuf.tile([NSUB, D], F16, tag="clo")
    nc.vector.tensor_tensor(out=clo[:], in0=cf[:], in1=chi[:],
                            op=mybir.AluOpType.subtract)
    chiclo = sbuf.tile([128, D], F16, tag="chiclo")
    nc.scalar.dma_start(chiclo[0:NSUB, :], chi[:])
    nc.scalar.dma_start(chiclo[NSUB:128, :], clo[:])

    # ---------------- SWDGE gathers: prep + trigger interleaved ------------
    # Traced after the table stores so the prep's table-read RAW dep (and the
    # trigger's store deps) are satisfied ~15us before the library is ready.
    tbl_ap = AP(tbl, 0, [[D, S], [1, D]])
    gsems = [ctx.enter_context(nc.semaphore(f"gsem{t}"))
             for t in range(NGATHER)]
    gts = []
    trig_insts = []
    for t in range(NGATHER):
        g_t = gpool.tile([128, 8, D], F16, tag=f"g{t}")
        pr = nc.gpsimd.dma_gather(
            out_ap=g_t[:], in_ap=tbl_ap, idxs_ap=idx16[:, 64 * t:64 * t + 64],
            num_idxs=1024, num_idxs_reg=1024, elem_size=D,
            prepare_only=True, sem=gsems[t])
        if trig_insts:
            add_dep_helper(pr.ins, trig_insts[-1].ins, sync=False,
                           reason="prep after previous trigger")
        trig = nc.gpsimd.trigger_dma(count=1)
        if trig_insts:
            add_dep_helper(trig.ins, trig_insts[-1].ins, sync=False,
                           reason="trigger order")
        trig_insts.append(trig)
        gts.append(g_t)
    for trig in trig_insts:
        for st in store_insts:
            add_dep_helper(trig.ins, st.ins, sync=True,
                           reason="gather transfers read table")

    # ---------------- combine: out = (onehot@C + tbl[e] - tbl[s]) * scale --
    for t in range(NGATHER):
        g_t = gts[t]
        for k in range(4):
            j = 4 * t + k
            ps = psum_c.tile([128, D], F32, tag="pc")
            nc.tensor.matmul(out=ps[:], lhsT=oh[:, 128 * j:128 * (j + 1)],
                             rhs=chiclo[:], start=True, stop=True)
            d_t = opool.tile([128, D], F32, tag="d")
            tt = nc.vector.tensor_tensor(out=d_t[:], in0=g_t[:, k, :],
                                         in1=g_t[:, 4 + k, :],
                                         op=mybir.AluOpType.subtract)
            tt._wait_ge(gsems[t], 16)
            add_dep_helper(tt.ins, trig_insts[t].ins, sync=False,
                           reason="consume after trigger")
            sm = opool.tile([128, D], F32, tag="sm")
            nc.vector.tensor_tensor(out=sm[:], in0=d_t[:], in1=ps[:],
                                    op=mybir.AluOpType.add)
            o_t = opool.tile([128, D], F32, tag="o")
            nc.scalar.mul(o_t[:], sm[:], scale[:, j:j + 1])
            nc.sync.dma_start(out[128 * j:128 * (j + 1), :], o_t[:])

    if dbg is not None:
        nc.sync.dma_start(dbg["tbl_sb"][:], tbl_sb[:])
        nc.sync.dma_start(dbg["oh"][:], oh[:])
        nc.sync.dma_start(dbg["cf"][:], cf[:])
        nc.sync.dma_start(dbg["scale"][:], scale[:])
        nc.sync.dma_start(dbg["t16"][:], t16[:])


def _make_consts(nc):
    blob = np.zeros((128, 576), dtype=np.float16)
    r = np.arange(128)
    blob[:, 0:128] = ((r[:, None] < r[None, :]) &
                      (r[:, None] // SUB == r[None, :] // SUB))
    blob[:, 128:256] = np.eye(128)
    blob[:, 256:384] = -np.eye(128)
    kp = np.arange(NSUB)
    true_b = SPB * (kp % 16) + kp // 16
    blob[0:64, 384:448] = (true_b[:, None] < np.arange(NSUB)[None, :])
    blob[0:1, 448:576] = 1.0
    k = np.arange(NSUB)
    thr2 = np.stack([float(SUB) * k, float(SUB) * (k + 1)],
                    axis=1).astype(np.float32)
    return {
        "blob": nc.inline_tensor(blob, name="c_blob"),
        "thr2": nc.inline_tensor(thr2, name="c_thr2"),
    }


def build_nc(debug_taps=False):
    nc = bacc.Bacc("TRN2", target_bir_lowering=False, debug=False,
                   dynamic_dma_scratch_size=2 ** 16)
    seq = nc.dram_tensor("seq", [S, D], F32, kind="ExternalInput")
    spans = nc.dram_tensor("spans", [N, 2], I32, kind="ExternalInput")
    maskw = nc.dram_tensor("maskw", [N], I32, kind="ExternalInput")
    out = nc.dram_tensor("out", [N, D], F32, kind="ExternalOutput")
    tbl = nc.dram_tensor("tbl", [S, D], F16, kind="Internal")
    consts = _make_consts(nc)
    dbg = None
    if debug_taps:
        dbg = {
            "tbl_sb": nc.dram_tensor("dbg_tbl", [128, NBLK, D], F16,
                                     kind="ExternalOutput").ap(),
            "oh": nc.dram_tensor("dbg_oh", [128, N], F16,
                                 kind="ExternalOutput").ap(),
            "cf": nc.dram_tensor("dbg_cf", [NSUB, D], F32,
                                 kind="ExternalOutput").ap(),
            "scale": nc.dram_tensor("dbg_scale", [128, NTILE], F32,
                                    kind="ExternalOutput").ap(),
            "t16": nc.dram_tensor("dbg_t16", [NSUB, D], F32,
                                  kind="ExternalOutput").ap(),
        }
    from contextlib import ExitStack
    with tile.TileContext(nc) as tc:
        with ExitStack() as ctx:
            build_kernel_body(tc, seq.ap(), spans.ap(), maskw.ap(), out.ap(),
                              tbl, consts, ctx, dbg=dbg)
    nc.compile()
    return nc


_NC_CACHE = None


def kernel(sequence_tensor: np.ndarray, span_indices: np.ndarray,
           span_indices_mask: np.ndarray) -> np.ndarray:
    global _NC_CACHE
    from concourse.bass_utils import run_bass_kernel_spmd

    if _NC_CACHE is None:
        _NC_CACHE = build_nc()
    nc = _NC_CACHE

    spans_i32 = np.ascontiguousarray(np.asarray(span_indices).astype(np.int32))
    mask_i32 = np.ascontiguousarray(np.asarray(span_indices_mask).astype(np.int32))
    seq_f32 = np.ascontiguousarray(sequence_tensor, dtype=np.float32)

    in_maps = [
        {"seq": seq_f32[b], "spans": spans_i32[b], "maskw": mask_i32[b]}
        for b in range(B)
    ]
    res = run_bass_kernel_spmd(nc, in_maps, core_ids=list(range(B)))
    return np.stack([r["out"] for r in res.results], axis=0)


# revision 24
# speedup vs baseline: 1.0228x; 1.0228x over previous
"""AverageSpanExtractor Trainium2 kernel (SWDGE gather + fp16 local table).

Math: out[b, n, :] = mean(seq[b, s_n:e_n, :]) * mask[b, n]

Per core (data-parallel over batch across 8 cores):
  1. Load seq [S=2048, D=512] f32, cast fp16.
  2. Per 128-token block: block-diag strict-upper matmul gives the 32-row
     SUB-BLOCK-local exclusive prefix L[r] = sum(seq[32*(r>>5) .. r)); cast
     fp16 and store to a DRAM table [2048, 512] fp16 (|L| < ~45 keeps fp16
     rounding well inside the 2e-2 gate).
  3. Gather tbl[e_n] and tbl[s_n] (raw indices -- exclusive table) with four
     512-idx SWDGE dma_gathers.  The mlp ucode library load (~28us) dominates
     the front; preps (~5us each) interleave with per-gather triggers so the
     transfers and combine overlap descriptor generation.
  4. Missing inter-sub-block offsets C[b] = sum of sub-block totals [0, b)
     are added exactly via a +-onehot fp16 matmul (onehot over e>>5 / s>>5
     from threshold compares on a rank-1 PSUM broadcast of the span indices)
     against hi/lo-split C vectors.
  5. Per span tile: PSUM = onehot@C;  DVE adds tbl[e]-tbl[s];  one ACT pass
     scales by mask/width; store.
"""

import numpy as np

import concourse.bacc as bacc
import concourse.bass as bass
import concourse.tile as tile
from concourse import mybir
from concourse.bass import AP
from concourse.library_config import mlp
from concourse.tile_rust import add_dep_helper

# Problem shape (hardcoded per contract).
B, S, D, N = 8, 2048, 512, 1024
NBLK = S // 128          # 16 token blocks (matmul granularity)
SUB = 32                 # sub-block rows (fp16 table locality)
NSUB = S // SUB          # 64 sub-blocks (offset-table entries)
SPB = 128 // SUB         # 4 sub-blocks per 128-block
NTILE = N // 128         # 8 span tiles
NGATHER = 4              # gather instructions (2 span tiles each)

F32 = mybir.dt.float32
F16 = mybir.dt.float16
I32 = mybir.dt.int32
I16 = mybir.dt.int16


def build_kernel_body(tc: tile.TileContext, seq: AP, spans: AP, maskw: AP,
                      out: AP, tbl, consts, ctx, dbg=None):
    nc = tc.nc
    const = ctx.enter_context(tc.tile_pool(name="const", bufs=1))
    sbuf = ctx.enter_context(tc.tile_pool(name="sbuf", bufs=1))
    opool = ctx.enter_context(tc.tile_pool(name="opool", bufs=3))
    gpool = ctx.enter_context(tc.tile_pool(name="gpool", bufs=1))
    psum_b = ctx.enter_context(tc.tile_pool(name="pb", bufs=2, space="PSUM"))
    psum_c = ctx.enter_context(tc.tile_pool(name="pc", bufs=2, space="PSUM"))
    psum_bc = ctx.enter_context(tc.tile_pool(name="pbc", bufs=1, space="PSUM"))

    # library load first: ~28us on the Pool engine, overlaps everything
    nc.gpsimd.load_library(mlp)

    # constants blob first (needed by the first matmul), then the seq halves
    # go straight onto the sync queue; all other tiny loads queue after them.
    cb = const.tile([128, 576], F16, tag="cb")
    nc.sync.dma_start(cb[:], consts["blob"].ap())
    u_strict = cb[:, 0:128]
    idp = cb[:, 128:256]
    u64s = cb[0:64, 384:448]
    ones1 = cb[0:1, 448:576]

    # ---------------- seq load + fp16 cast + local cumsum + table store ----
    # (seq chunk DMAs queue behind the tiny loads above: first chunk lands
    # ~4.5us, last ~13.5us; stores complete ~17us -- all before the library)
    store_insts = []
    xbig = sbuf.tile([128, NBLK, D], F32, tag="xbig")
    xf = sbuf.tile([128, NBLK, D], F16, tag="xf")
    tbl_sb = sbuf.tile([128, NBLK, D], F16, tag="tbl_sb")
    for h in range(2):
        sl = (slice(None), slice(8 * h, 8 * h + 8), slice(None))
        nc.sync.dma_start(
            xbig[sl],
            seq[1024 * h:1024 * (h + 1), :].rearrange("(j p) d -> p j d", p=128))
        for q in range(2 * h, 2 * h + 2):
            slq = (slice(None), slice(4 * q, 4 * q + 4), slice(None))
            nc.vector.tensor_copy(xf[slq], xbig[slq])
            for b in range(4 * q, 4 * q + 4):
                pl = psum_b.tile([128, D], F32, tag="pb")
                nc.tensor.matmul(out=pl[:], lhsT=u_strict, rhs=xf[:, b, :],
                                 start=True, stop=True)
                if b % 2 == 0:
                    nc.scalar.copy(tbl_sb[:, b, :], pl[:])
                else:
                    nc.vector.tensor_copy(tbl_sb[:, b, :], pl[:])
            # store 4 blocks (512 rows) to the DRAM table, scalar queue
            store_insts.append(nc.scalar.dma_start(
                AP(tbl, 512 * D * q, [[D, 128], [128 * D, 4], [1, D]]),
                tbl_sb[:, 4 * q:4 * q + 4, :]))

    # ---------------- gather idx list + onehot inputs (post-build trace) ---
    # a32[p, 32t + u (+16)] = (e|s) of span 256t + 16u + p; these tiny sync
    # DMAs queue behind the seq chunks and land ~14us -- needed only at the
    # ~35us library-ready point.
    st_pj = sbuf.tile([128, NTILE], I32, tag="st_pj")
    en_pj = sbuf.tile([128, NTILE], I32, tag="en_pj")
    mk_pj = sbuf.tile([128, NTILE], I32, tag="mk_pj")
    nc.sync.dma_start(st_pj[:], AP(spans.tensor, 0, [[2, 128], [256, NTILE]]))
    nc.sync.dma_start(en_pj[:], AP(spans.tensor, 1, [[2, 128], [256, NTILE]]))
    nc.sync.dma_start(mk_pj[:], AP(maskw.tensor, 0, [[1, 128], [128, NTILE]]))
    thr2 = const.tile([NSUB, 2], F32, tag="thr2")
    nc.sync.dma_start(thr2[:], consts["thr2"].ap())
    thr_lo = thr2[:, 0:1]
    thr_hi = thr2[:, 1:2]
    aE = sbuf.tile([16, 64], I32, tag="aE")
    aS = sbuf.tile([16, 64], I32, tag="aS")
    nc.sync.dma_start(aE[:], AP(spans.tensor, 1, [[2, 16], [32, 64]]))
    nc.sync.dma_start(aS[:], AP(spans.tensor, 0, [[2, 16], [32, 64]]))
    idx16 = sbuf.tile([128, 128], I16, tag="idx16")
    i16v = idx16[0:16, :].rearrange("p (t f u) -> p t f u", t=NGATHER, f=2)
    nc.vector.tensor_copy(i16v[:, :, 0, :],
                          aE[:].rearrange("p (t u) -> p t u", t=NGATHER))
    nc.vector.tensor_copy(i16v[:, :, 1, :],
                          aS[:].rearrange("p (t u) -> p t u", t=NGATHER))
    nc.sync.dma_start(idx16[16:32, :], idx16[0:16, :])
    nc.sync.dma_start(idx16[32:64, :], idx16[0:32, :])
    nc.sync.dma_start(idx16[64:128, :], idx16[0:64, :])

    # span indices to fp16, PE transpose, fold to a [1, 2048] row, rank-1
    # broadcast into PSUM (compare ops read it later, after the build)
    es16 = sbuf.tile([128, 2 * NTILE], F16, tag="es16")
    nc.vector.tensor_copy(es16[:, 0:NTILE], en_pj[:])
    nc.vector.tensor_copy(es16[:, NTILE:2 * NTILE], st_pj[:])
    psT = psum_c.tile([2 * NTILE, 128], F16, tag="pc")
    nc.tensor.transpose(out=psT[:], in_=es16[:], identity=idp)
    esT = sbuf.tile([2 * NTILE, 128], F16, tag="esT")
    nc.vector.tensor_copy(esT[:], psT[:])
    esrow = sbuf.tile([1, 2 * N], F16, tag="esrow")
    nc.sync.dma_start(esrow[0:1, 0:N].rearrange("p (j c) -> p j c", j=NTILE),
                      esT[0:NTILE, :])
    nc.sync.dma_start(esrow[0:1, N:2 * N].rearrange("p (j c) -> p j c", j=NTILE),
                      esT[NTILE:2 * NTILE, :])
    bce = psum_bc.tile([128, N], F32, tag="bc")
    nc.tensor.matmul(out=bce[:, 0:D], lhsT=ones1, rhs=esrow[0:1, 0:D],
                     start=True, stop=True)
    nc.tensor.matmul(out=bce[:, D:N], lhsT=ones1, rhs=esrow[0:1, D:N],
                     start=True, stop=True)
    bcs = psum_bc.tile([128, N], F32, tag="bc2")
    nc.tensor.matmul(out=bcs[:, 0:D], lhsT=ones1, rhs=esrow[0:1, N:N + D],
                     start=True, stop=True)
    nc.tensor.matmul(out=bcs[:, D:N], lhsT=ones1, rhs=esrow[0:1, N + D:2 * N],
                     start=True, stop=True)

    # ---------------- onehot rows (DVE compares, after the build casts) ----
    cme = sbuf.tile([NSUB, N], F32, tag="cme")
    nc.vector.tensor_scalar(out=cme[:], in0=bce[0:NSUB, :], scalar1=thr_lo,
                            scalar2=None, op0=mybir.AluOpType.is_ge)
    tmp = sbuf.tile([NSUB, N], F32, tag="tmp")
    nc.vector.tensor_scalar(out=tmp[:], in0=bce[0:NSUB, :], scalar1=thr_hi,
                            scalar2=None, op0=mybir.AluOpType.is_ge)
    nc.vector.tensor_tensor(out=cme[:], in0=cme[:], in1=tmp[:],
                            op=mybir.AluOpType.subtract)
    cms = sbuf.tile([NSUB, N], F32, tag="cms")
    nc.vector.tensor_scalar(out=cms[:], in0=bcs[0:NSUB, :], scalar1=thr_lo,
                            scalar2=None, op0=mybir.AluOpType.is_ge)
    nc.vector.tensor_scalar(out=tmp[:], in0=bcs[0:NSUB, :], scalar1=thr_hi,
                            scalar2=None, op0=mybir.AluOpType.is_ge)
    nc.vector.tensor_tensor(out=cms[:], in0=cms[:], in1=tmp[:],
                            op=mybir.AluOpType.subtract)
    oh = sbuf.tile([128, N], F16, tag="oh")
    nc.vector.tensor_tensor(out=oh[0:NSUB, :], in0=cme[:], in1=cms[:],
                            op=mybir.AluOpType.subtract)
    nc.scalar.dma_start(oh[NSUB:128, :], oh[0:NSUB, :])

    # per-span scale = mask / width
    w_i = sbuf.tile([128, NTILE], I32, tag="w_i")
    nc.vector.tensor_tensor(out=w_i[:], in0=en_pj[:], in1=st_pj[:],
                            op=mybir.AluOpType.subtract)
    w_f = sbuf.tile([128, NTILE], F32, tag="w_f")
    nc.vector.tensor_copy(w_f[:], w_i[:])
    r_f = sbuf.tile([128, NTILE], F32, tag="r_f")
    nc.vector.reciprocal(r_f[:], w_f[:])
    m_f = sbuf.tile([128, NTILE], F32, tag="m_f")
    nc.vector.tensor_copy(m_f[:], mk_pj[:])
    scale = sbuf.tile([128, NTILE], F32, tag="scale")
    nc.vector.tensor_tensor(out=scale[:], in0=r_f[:], in1=m_f[:],
                            op=mybir.AluOpType.mult)

    # ---------------- sub-block totals -> offsets C ------------------------
    # T_b = L[32b + 31] + x[32b + 31]; PERMUTED row order k' = 16i + blk,
    # matching u64s whose rows are permuted (columns in true order b).
    t16f = sbuf.tile([NSUB, D], F16, tag="t16f")
    xrow = sbuf.tile([NSUB, D], F16, tag="xrow")
    for i in range(SPB):
        p = SUB * i + SUB - 1
        nc.scalar.dma_start(t16f[16 * i:16 * (i + 1), :], tbl_sb[p:p + 1, :, :])
        nc.scalar.dma_start(xrow[16 * i:16 * (i + 1), :], xf[p:p + 1, :, :])
    t16 = sbuf.tile([NSUB, D], F32, tag="t16")
    nc.vector.tensor_tensor(out=t16[:], in0=t16f[:], in1=xrow[:],
                            op=mybir.AluOpType.add)
    th = sbuf.tile([NSUB, D], F16, tag="th")
    nc.vector.tensor_copy(th[:], t16[:])
    tl = sbuf.tile([NSUB, D], F16, tag="tl")
    nc.vector.tensor_tensor(out=tl[:], in0=t16[:], in1=th[:],
                            op=mybir.AluOpType.subtract)
    poff = psum_c.tile([NSUB, D], F32, tag="pc")
    nc.tensor.matmul(out=poff[:], lhsT=u64s, rhs=th[:], start=True, stop=False)
    nc.tensor.matmul(out=poff[:], lhsT=u64s, rhs=tl[:], start=False, stop=True)
    cf = sbuf.tile([NSUB, D], F32, tag="cf")
    nc.vector.tensor_copy(cf[:], poff[:])
    chi = sbuf.tile([NSUB, D], F16, tag="chi")
    nc.vector.tensor_copy(chi[:], cf[:])
    clo = sbuf.tile([NSUB, D], F16, tag="clo")
    nc.vector.tensor_tensor(out=clo[:], in0=cf[:], in1=chi[:],
                            op=mybir.AluOpType.subtract)
    chiclo = sbuf.tile([128, D], F16, tag="chiclo")
    nc.scalar.dma_start(chiclo[0:NSUB, :], chi[:])
    nc.scalar.dma_start(chiclo[NSUB:128, :], clo[:])

    # ---------------- SWDGE gathers: prep + trigger interleaved ------------
    # Traced after the table stores so the prep's table-read RAW dep (and the
    # trigger's store deps) are satisfied ~15us before the library is ready.
    tbl_ap = AP(tbl, 0, [[D, S], [1, D]])
    gsems = [ctx.enter_context(nc.semaphore(f"gsem{t}"))
             for t in range(NGATHER)]
    gts = []
    trig_insts = []
    for t in range(NGATHER):
        g_t = gpool.tile([128, 4, D], F16, tag=f"g{t}")
        pr = nc.gpsimd.dma_gather(
            out_ap=g_t[:], in_ap=tbl_ap, idxs_ap=idx16[:, 32 * t:32 * t + 32],
            num_idxs=512, num_idxs_reg=512, elem_size=D,
            prepare_only=True, sem=gsems[t])
        if trig_insts:
            add_dep_helper(pr.ins, trig_insts[-1].ins, sync=False,
                           reason="prep after previous trigger")
        trig = nc.gpsimd.trigger_dma(count=1)
        if trig_insts:
            add_dep_helper(trig.ins, trig_insts[-1].ins, sync=False,
                           reason="trigger order")
        trig_insts.append(trig)
        gts.append(g_t)
    for trig in trig_insts:
        for st in store_insts:
            add_dep_helper(trig.ins, st.ins, sync=True,
                           reason="gather transfers read table")

    # ---------------- combine: out = (onehot@C + tbl[e] - tbl[s]) * scale --
    for t in range(NGATHER):
        g_t = gts[t]
        for k in range(2):
            j = 2 * t + k
            ps = psum_c.tile([128, D], F32, tag="pc")
            nc.tensor.matmul(out=ps[:], lhsT=oh[:, 128 * j:128 * (j + 1)],
                             rhs=chiclo[:], start=True, stop=True)
            d_t = opool.tile([128, D], F32, tag="d")
            tt = nc.vector.tensor_tensor(out=d_t[:], in0=g_t[:, k, :],
                                         in1=g_t[:, 2 + k, :],
                                         op=mybir.AluOpType.subtract)
            tt._wait_ge(gsems[t], 16)
            add_dep_helper(tt.ins, trig_insts[t].ins, sync=False,
                           reason="consume after trigger")
            sm = opool.tile([128, D], F32, tag="sm")
            nc.vector.tensor_tensor(out=sm[:], in0=d_t[:], in1=ps[:],
                                    op=mybir.AluOpType.add)
            o_t = opool.tile([128, D], F32, tag="o")
            nc.scalar.mul(o_t[:], sm[:], scale[:, j:j + 1])
            nc.sync.dma_start(out[128 * j:128 * (j + 1), :], o_t[:])

    if dbg is not None:
        nc.sync.dma_start(dbg["tbl_sb"][:], tbl_sb[:])
        nc.sync.dma_start(dbg["oh"][:], oh[:])
        nc.sync.dma_start(dbg["cf"][:], cf[:])
        nc.sync.dma_start(dbg["scale"][:], scale[:])
        nc.sync.dma_start(dbg["t16"][:], t16[:])


def _make_consts(nc):
    blob = np.zeros((128, 576), dtype=np.float16)
    r = np.arange(128)
    blob[:, 0:128] = ((r[:, None] < r[None, :]) &
                      (r[:, None] // SUB == r[None, :] // SUB))
    blob[:, 128:256] = np.eye(128)
    blob[:, 256:384] = -np.eye(128)
    kp = np.arange(NSUB)
    true_b = SPB * (kp % 16) + kp // 16
    blob[0:64, 384:448] = (true_b[:, None] < np.arange(NSUB)[None, :])
    blob[0:1, 448:576] = 1.0
    k = np.arange(NSUB)
    thr2 = np.stack([float(SUB) * k, float(SUB) * (k + 1)],
                    axis=1).astype(np.float32)
    return {
        "blob": nc.inline_tensor(blob, name="c_blob"),
        "thr2": nc.inline_tensor(thr2, name="c_thr2"),
    }


def build_nc(debug_taps=False):
    nc = bacc.Bacc("TRN2", target_bir_lowering=False, debug=False,
                   dynamic_dma_scratch_size=2 ** 16)
    seq = nc.dram_tensor("seq", [S, D], F32, kind="ExternalInput")
    spans = nc.dram_tensor("spans", [N, 2], I32, kind="ExternalInput")
    maskw = nc.dram_tensor("maskw", [N], I32, kind="ExternalInput")
    out = nc.dram_tensor("out", [N, D], F32, kind="ExternalOutput")
    tbl = nc.dram_tensor("tbl", [S, D], F16, kind="Internal")
    consts = _make_consts(nc)
    dbg = None
    if debug_taps:
        dbg = {
            "tbl_sb": nc.dram_tensor("dbg_tbl", [128, NBLK, D], F16,
                                     kind="ExternalOutput").ap(),
            "oh": nc.dram_tensor("dbg_oh", [128, N], F16,
                                 kind="ExternalOutput").ap(),
            "cf": nc.dram_tensor("dbg_cf", [NSUB, D], F32,
                                 kind="ExternalOutput").ap(),
            "scale": nc.dram_tensor("dbg_scale", [128, NTILE], F32,
                                    kind="ExternalOutput").ap(),
            "t16": nc.dram_tensor("dbg_t16", [NSUB, D], F32,
                                  kind="ExternalOutput").ap(),
        }
    from contextlib import ExitStack
    with tile.TileContext(nc) as tc:
        with ExitStack() as ctx:
            build_kernel_body(tc, seq.ap(), spans.ap(), maskw.ap(), out.ap(),
                              tbl, consts, ctx, dbg=dbg)
    nc.compile()
    return nc


_NC_CACHE = None


def kernel(sequence_tensor: np.ndarray, span_indices: np.ndarray,
           span_indices_mask: np.ndarray) -> np.ndarray:
    global _NC_CACHE
    from concourse.bass_utils import run_bass_kernel_spmd

    if _NC_CACHE is None:
        _NC_CACHE = build_nc()
    nc = _NC_CACHE

    spans_i32 = np.ascontiguousarray(np.asarray(span_indices).astype(np.int32))
    mask_i32 = np.ascontiguousarray(np.asarray(span_indices_mask).astype(np.int32))
    seq_f32 = np.ascontiguousarray(sequence_tensor, dtype=np.float32)

    in_maps = [
        {"seq": seq_f32[b], "spans": spans_i32[b], "maskw": mask_i32[b]}
        for b in range(B)
    ]
    res = run_bass_kernel_spmd(nc, in_maps, core_ids=list(range(B)))
    return np.stack([r["out"] for r in res.results], axis=0)
